# revision 2
# baseline (speedup 1.0000x reference)
"""Multi-head attention (B=2, S=2048, E=1024, H=16) on 8 TRN2 NeuronCores.

Sharding: batch x head-quarter. Core c handles batch c//4 and heads
{4*(c%4) .. 4*(c%4)+3} end to end (QKV slice, attention, row-parallel slice
of out_proj over its 4 heads' 256 contraction dims). Each core returns a
bf16 partial [2048, 1024]; the host sums groups of 4 in fp32 and adds b_out.

vs the head-only sharding this halves per-core input DMA (x for one batch
only) and halves partial-output DMA again via bf16, and the 4-head grouping
lets out_proj contract K=128 per matmul (two heads stacked per partition
range) instead of K=64.

Per-core device program (identical on all cores; only input data differs):
  phase 1: qkvT[f, t] = sum_E w_inT[E, f] * xT[E, t] + b_in; 6 feature
           chunks of 128 = (q | k | v) x 2 head-pairs; q,k stay
           feature-major in qkT [128, hp, qk, 2048]; v is PE-transposed
           into vT [128 kpos, hp, kc, 130] with ones cols 64/129 (softmax
           sums ride along PV for free).
  phase 2: per (qb of 1024 q, hp): scoresT psum [128 k, 1024 q] per
           (kc, hi) with both heads of the pair row-packed concurrently
           (tile_position=(64*hi, 0)); ACT exp(0.125*s) -> es bf16.
  phase 3: PV: psum[65, 512] = [v | 1].T @ es accumulated over 16 kc
           (row 64 = softmax sums); normalize via reciprocal + gpsimd
           partition_broadcast + DVE multiply into attn [128, hp, 1024]
           (head-in-pair hi stacked across partition halves).
  phase 4: out_proj: psum[128 t, 512 e] accumulates the 2 head-pair
           K=128 matmuls; evict bf16 + DMA partial out.
"""
import sys

sys.path.insert(0, "/opt/trn_rl_repo")
import numpy as np
import ml_dtypes
import concourse.bass as bass
import concourse.mybir as mybir
import concourse.tile as tile
from concourse import bacc
from concourse.bass_utils import run_bass_kernel_spmd
from concourse.masks import make_identity

P = 128
B = 2
S = 2048          # tokens per core (= one batch)
E = 1024
H = 16
D = 64            # head dim
HC = 4            # heads per core
NCORES = 8
EC = E // P       # 8 contraction chunks for QKV
NCH = 3 * HC * D // P   # 6 feature chunks (q|k|v x 2 head pairs)
QB = 1024         # q block size
NQB = S // QB     # 2 q blocks
KC = S // P       # 16 k chunks
TB = 512          # token block for streaming xT
NTB = S // TB     # 4
QQ = 512          # psum bank width

F32 = mybir.dt.float32
BF16 = mybir.dt.bfloat16

_COMPILED = None


def build(repeat=1):
    nc = bacc.Bacc(None, target_bir_lowering=False)
    xT_d = nc.dram_tensor("xT", [P, EC, S], BF16, kind="ExternalInput")
    w_inT_d = nc.dram_tensor("w_inT", [P, EC, NCH * P], BF16, kind="ExternalInput")
    b_in_d = nc.dram_tensor("b_in", [P, NCH], F32, kind="ExternalInput")
    w_outT_d = nc.dram_tensor("w_outT", [P, 2, E], BF16, kind="ExternalInput")
    out_d = nc.dram_tensor("out", [S // P, P, E], BF16, kind="ExternalOutput")

    with tile.TileContext(nc) as tc:
        with (
            tc.tile_pool(name="const", bufs=1) as const,
            tc.tile_pool(name="main", bufs=1) as main,
            tc.tile_pool(name="attn_p", bufs=2) as attn_p,
            tc.tile_pool(name="outp", bufs=3) as outp,
            tc.tile_pool(name="small", bufs=2) as small,
            tc.tile_pool(name="qkv_in", bufs=2) as qkv_in,
            tc.tile_pool(name="es_pool", bufs=12) as es_pool,
            tc.tile_pool(name="psum", bufs=1, space="PSUM") as psum,
        ):
            identity = const.tile([P, P], BF16)
            make_identity(nc, identity)
            b_in_sb = const.tile([P, NCH], F32)
            nc.sync.dma_start(b_in_sb[:], b_in_d[:])
            w_outT_sb = const.tile([P, 2, E], BF16)
            nc.sync.dma_start(w_outT_sb[:], w_outT_d[:])
            w_inT_sb = const.tile([P, EC, NCH * P], BF16)
            nc.sync.dma_start(w_inT_sb[:], w_inT_d[:])

            # q,k feature-major: [128 = hi*64+d, hp, (q|k), token]
            qkT = main.tile([P, 2, 2, S], BF16)          # 16 KB/partition
            # v token-major: [128 kpos, hp, kc, 130]; cols 64/129 are ones
            vT = main.tile([P, 2, KC, 130], BF16)        # ~8.3 KB/partition
            nc.vector.memset(vT[:, :, :, 64:65], 1.0)
            nc.vector.memset(vT[:, :, :, 129:130], 1.0)

            for _rep in range(repeat):
                def emit_tb(tbi):
                    xt = qkv_in.tile([P, EC, TB], BF16, name="xt")
                    nc.sync.dma_start(xt[:], xT_d[:, :, tbi * TB:(tbi + 1) * TB])
                    for hp in range(2):
                        v_sb = qkv_in.tile([P, TB], BF16, name="vsb")
                        for j in range(3):          # q, k, v
                            ch = hp * 3 + j
                            acc = psum.tile([P, TB], F32, name="qkvp", bufs=2)
                            for ec in range(EC):
                                nc.tensor.matmul(
                                    acc[:],
                                    w_inT_sb[:, ec, ch * P:(ch + 1) * P],
                                    xt[:, ec, :],
                                    start=(ec == 0),
                                    stop=(ec == EC - 1),
                                )
                            nc.vector.tensor_scalar(
                                out=(qkT[:, hp, j, tbi * TB:(tbi + 1) * TB]
                                     if j < 2 else v_sb[:]),
                                in0=acc[:],
                                scalar1=b_in_sb[:, ch:ch + 1],
                                scalar2=None,
                                op0=mybir.AluOpType.add,
                            )
                        for kci in range(TB // P):
                            kc = tbi * (TB // P) + kci
                            tp = psum.tile([P, P], BF16, name="qkvp", bufs=2)
                            nc.tensor.transpose(
                                tp[:], v_sb[:, kci * P:(kci + 1) * P], identity[:]
                            )
                            nc.vector.tensor_copy(vT[:, hp, kc, 0:64], tp[:, 0:64])
                            nc.vector.tensor_copy(vT[:, hp, kc, 65:129], tp[:, 64:128])

                def emit_scores(qb, hp, kcs, es_tiles):
                    q0 = qb * QB
                    for kc in kcs:
                        for hi in range(2):
                            sc = psum.tile([P, QB], F32, name="sc", bufs=2)
                            for qh in range(QB // QQ):
                                nc.tensor.matmul(
                                    sc[:, qh * QQ:(qh + 1) * QQ],
                                    qkT[hi * D:(hi + 1) * D, hp, 1,
                                        kc * P:(kc + 1) * P],
                                    qkT[hi * D:(hi + 1) * D, hp, 0,
                                        q0 + qh * QQ:q0 + (qh + 1) * QQ],
                                    start=True, stop=True,
                                    tile_position=(hi * D, 0),
                                )
                            if kc % 4 == 0:
                                es_tiles[(hi, kc // 4)] = es_pool.tile(
                                    [P, 4, QB], BF16, name="es"
                                )
                            nc.scalar.activation(
                                es_tiles[(hi, kc // 4)][:, kc % 4, :],
                                sc[:],
                                mybir.ActivationFunctionType.Exp,
                                scale=0.125,
                            )

                def emit_pv(hp, es_tiles, attn):
                    for hi in range(2):
                        for qh in range(QB // QQ):
                            pv = psum.tile([65, QQ], F32, name="pv", bufs=2)
                            for kc in range(KC):
                                nc.tensor.matmul(
                                    pv[:],
                                    vT[:, hp, kc, hi * 65:(hi + 1) * 65],
                                    es_tiles[(hi, kc // 4)][:, kc % 4,
                                                            qh * QQ:(qh + 1) * QQ],
                                    start=(kc == 0),
                                    stop=(kc == KC - 1),
                                )
                            inv = small.tile([1, QQ], F32, name="inv")
                            nc.vector.reciprocal(inv[:], pv[64:65, :])
                            inv_b = small.tile([D, QQ], F32, name="invb")
                            nc.gpsimd.partition_broadcast(inv_b[:], inv[:], channels=D)
                            nc.vector.tensor_tensor(
                                out=attn[hi * D:(hi + 1) * D, hp,
                                         qh * QQ:(qh + 1) * QQ],
                                in0=pv[0:64, :],
                                in1=inv_b[:],
                                op=mybir.AluOpType.mult,
                            )

                # ---- QKV with early score start ----
                emit_tb(0)
                emit_tb(1)
                es_q0 = {}
                emit_scores(0, 0, range(0, 4), es_q0)
                emit_tb(2)
                emit_scores(0, 0, range(4, 8), es_q0)
                emit_tb(3)
                emit_scores(0, 0, range(8, 16), es_q0)

                for qb in range(NQB):
                    q0 = qb * QB
                    attn = attn_p.tile([P, 2, QB], BF16, name="attn")
                    for hp in range(2):
                        if hp == 0 and qb == 0:
                            es_tiles = es_q0
                        else:
                            es_tiles = {}
                            emit_scores(qb, hp, range(KC), es_tiles)
                        emit_pv(hp, es_tiles, attn)

                    # ---- out_proj for this q block ----
                    for tci in range(QB // P):
                        tc_g = (q0 + tci * P) // P
                        out_sb = outp.tile([P, E], BF16, name="osb")
                        for eb in range(E // QQ):
                            op = psum.tile([P, QQ], F32, name="pv", bufs=2)
                            for hp in range(2):
                                nc.tensor.matmul(
                                    op[:],
                                    attn[:, hp, tci * P:(tci + 1) * P],
                                    w_outT_sb[:, hp, eb * QQ:(eb + 1) * QQ],
                                    start=(hp == 0),
                                    stop=(hp == 1),
                                )
                            nc.vector.tensor_copy(
                                out_sb[:, eb * QQ:(eb + 1) * QQ], op[:]
                            )
                        nc.sync.dma_start(out_d[tc_g], out_sb[:])

    nc.compile()
    return nc


def _prep_inputs(x, w_in, b_in, w_out):
    x = np.ascontiguousarray(np.asarray(x, dtype=np.float32))
    w_in = np.asarray(w_in, dtype=np.float32)
    b_in = np.asarray(b_in, dtype=np.float32)
    w_out = np.asarray(w_out, dtype=np.float32)

    xT_b = []
    for b in range(B):
        xT_b.append(np.ascontiguousarray(
            x[b].T.reshape(EC, P, S).transpose(1, 0, 2)
        ).astype(ml_dtypes.bfloat16))          # [128, EC, S]

    in_maps = []
    for c in range(NCORES):
        b = c // 4
        hg = c % 4
        h0 = hg * HC                            # first head of this core
        # 6 feature chunks: (hp, j) -> rows of w_in
        rows = []
        brows = []
        for hp in range(2):
            for j in range(3):                  # q, k, v
                for hi in range(2):
                    h = h0 + hp * 2 + hi
                    r0 = j * E + h * D
                    rows.append(w_in[r0:r0 + D])
                    brows.append(b_in[r0:r0 + D])
        rows = np.concatenate(rows)             # [768, 1024]
        w_inT_c = np.ascontiguousarray(
            rows.T.reshape(EC, P, NCH * P).transpose(1, 0, 2)
        ).astype(ml_dtypes.bfloat16)            # [128, EC, 768]
        b_c = np.ascontiguousarray(
            np.concatenate(brows).reshape(NCH, P).T
        )                                       # [128, 6]
        # w_outT [128 = hi*64+d, hp, E]
        w_outT_c = np.empty((P, 2, E), dtype=np.float32)
        for hp in range(2):
            for hi in range(2):
                h = h0 + hp * 2 + hi
                w_outT_c[hi * D:(hi + 1) * D, hp] = w_out[:, h * D:(h + 1) * D].T
        w_outT_c = w_outT_c.astype(ml_dtypes.bfloat16)
        in_maps.append({
            "xT": xT_b[b],
            "w_inT": w_inT_c,
            "b_in": b_c,
            "w_outT": w_outT_c,
        })
    return in_maps


def kernel(x, w_in, b_in, w_out, b_out, _trace=False):
    global _COMPILED
    if _COMPILED is None:
        _COMPILED = build()
    nc = _COMPILED

    in_maps = _prep_inputs(x, w_in, b_in, w_out)
    res = run_bass_kernel_spmd(
        nc, in_maps, core_ids=list(range(NCORES)), trace=_trace
    )
    out = np.zeros((B, S, E), dtype=np.float32)
    for c in range(NCORES):
        b = c // 4
        out[b] += res.results[c]["out"].astype(np.float32).reshape(S, E)
    out += np.asarray(b_out, dtype=np.float32)
    if _trace:
        return out, res
    return out


# revision 4
# speedup vs baseline: 1.3942x; 1.3942x over previous
"""Multi-head attention (B=2, S=2048, E=1024, H=16) on 8 TRN2 NeuronCores.

Sharding: batch x head-quarter. Core c handles batch c//4 and heads
{4*(c%4) .. 4*(c%4)+3} end to end (QKV slice, attention, row-parallel slice
of out_proj over its 4 heads' 256 contraction dims). Each core returns a
bf16 partial [2048, 1024]; the host sums groups of 4 in fp32 and adds b_out.

The device program is software-pipelined around the ACT engine (the exp of
134M/8 attention weights is the largest single-engine load): the kernel is
split into 8 phases (4 q-blocks of 512 x 2 head-pairs); scores for phase
i+1 are emitted before PV of phase i so ACT always has exp work queued, and
the next repeat's QKV runs inside the current repeat's tail (qk tiles are
read-complete by then; vT is double-buffered).

Per-phase structure:
  scores: psum [128 k, 2 hi, 512 q] per kc; the head pair is row-packed
          (tile_position=(64*hi, 0)) so both heads' score matmuls run
          concurrently on the PE; one exp per (kc, hi) -> es bf16.
  PV:     psum [65, 512] per hi = [v | 1].T @ es accumulated over 16 kc
          (row 64 = softmax sums; es tiles are consumed kc-major and free
          early); normalize via reciprocal + gpsimd partition_broadcast +
          DVE multiply into attn [128, hp, 512] (hi stacked across
          partition halves).
  out_proj (per q-block): psum [128 t, 512 e] accumulates 2 K=128
          head-pair matmuls; evict bf16 + DMA partial out.
"""
import sys

sys.path.insert(0, "/opt/trn_rl_repo")
import numpy as np
import ml_dtypes
import concourse.bass as bass
import concourse.mybir as mybir
import concourse.tile as tile
from concourse import bacc
from concourse.bass_utils import run_bass_kernel_spmd
from concourse.masks import make_identity

P = 128
B = 2
S = 2048          # tokens per core (= one batch)
E = 1024
H = 16
D = 64            # head dim
HC = 4            # heads per core
NCORES = 8
EC = E // P       # 8 contraction chunks for QKV
NCH = 3 * HC * D // P   # 6 feature chunks (q|k|v x 2 head pairs)
QB = 512          # q block (phase) size
NQB = S // QB     # 4 q blocks
KC = S // P       # 16 k chunks
TB = 512          # token block for streaming xT
NTB = S // TB     # 4
EKC = 2           # kc chunks grouped per es tile

F32 = mybir.dt.float32
BF16 = mybir.dt.bfloat16

_COMPILED = None


def build(repeat=1):
    nc = bacc.Bacc(None, target_bir_lowering=False)
    xT_d = nc.dram_tensor("xT", [P, EC, S], BF16, kind="ExternalInput")
    w_inT_d = nc.dram_tensor("w_inT", [P, EC, NCH * P], BF16, kind="ExternalInput")
    b_in_d = nc.dram_tensor("b_in", [P, NCH], F32, kind="ExternalInput")
    w_outT_d = nc.dram_tensor("w_outT", [P, 2, E], BF16, kind="ExternalInput")
    out_d = nc.dram_tensor("out", [S // P, P, E], BF16, kind="ExternalOutput")

    with tile.TileContext(nc) as tc:
        with (
            tc.tile_pool(name="const", bufs=1) as const,
            tc.tile_pool(name="qk_p", bufs=1) as qk_p,
            tc.tile_pool(name="vt_p", bufs=2) as vt_p,
            tc.tile_pool(name="attn_p", bufs=2) as attn_p,
            tc.tile_pool(name="outp", bufs=3) as outp,
            tc.tile_pool(name="small", bufs=2) as small,
            tc.tile_pool(name="qkv_in", bufs=2) as qkv_in,
            tc.tile_pool(name="es_pool", bufs=14) as es_pool,
            tc.tile_pool(name="psum", bufs=1, space="PSUM") as psum,
        ):
            identity = const.tile([P, P], BF16)
            make_identity(nc, identity)
            b_in_sb = const.tile([P, NCH], F32)
            nc.sync.dma_start(b_in_sb[:], b_in_d[:])
            w_outT_sb = const.tile([P, 2, E], BF16)
            nc.sync.dma_start(w_outT_sb[:], w_outT_d[:])
            w_inT_sb = const.tile([P, EC, NCH * P], BF16)
            nc.sync.dma_start(w_inT_sb[:], w_inT_d[:])

            # q,k feature-major, one tile per (hp, q|k): [128 = hi*64+d, token]
            qk = [[qk_p.tile([P, S], BF16, name=f"qk{hp}{j}")
                   for j in range(2)] for hp in range(2)]

            def emit_tb(tbi, vT):
                xt = qkv_in.tile([P, EC, TB], BF16, name="xt")
                nc.sync.dma_start(xt[:], xT_d[:, :, tbi * TB:(tbi + 1) * TB])
                for hp in range(2):
                    v_sb = qkv_in.tile([P, TB], BF16, name="vsb")
                    for j in range(3):          # q, k, v
                        ch = hp * 3 + j
                        acc = psum.tile([P, TB], F32, name="work", bufs=2)
                        for ec in range(EC):
                            nc.tensor.matmul(
                                acc[:],
                                w_inT_sb[:, ec, ch * P:(ch + 1) * P],
                                xt[:, ec, :],
                                start=(ec == 0),
                                stop=(ec == EC - 1),
                            )
                        nc.vector.tensor_scalar(
                            out=(qk[hp][j][:, tbi * TB:(tbi + 1) * TB]
                                 if j < 2 else v_sb[:]),
                            in0=acc[:],
                            scalar1=b_in_sb[:, ch:ch + 1],
                            scalar2=None,
                            op0=mybir.AluOpType.add,
                        )
                    for kci in range(TB // P):
                        kc = tbi * (TB // P) + kci
                        tp = psum.tile([P, P], BF16, name="work", bufs=2)
                        nc.tensor.transpose(
                            tp[:], v_sb[:, kci * P:(kci + 1) * P], identity[:]
                        )
                        nc.vector.tensor_copy(vT[:, hp, kc, 0:64], tp[:, 0:64])
                        nc.vector.tensor_copy(vT[:, hp, kc, 65:129], tp[:, 64:128])

            def emit_scores(qb, hp, kcs, es_tiles):
                q0 = qb * QB
                for kc in kcs:
                    sc = psum.tile([P, 2, QB], F32, name="sc", bufs=2)
                    for hi in range(2):
                        nc.tensor.matmul(
                            sc[:, hi, :],
                            qk[hp][1][hi * D:(hi + 1) * D, kc * P:(kc + 1) * P],
                            qk[hp][0][hi * D:(hi + 1) * D, q0:q0 + QB],
                            start=True, stop=True,
                            tile_position=(hi * D, 0),
                        )
                    if kc % EKC == 0:
                        es_tiles[kc // EKC] = es_pool.tile(
                            [P, EKC, 2, QB], BF16, name="es"
                        )
                    nc.scalar.activation(
                        es_tiles[kc // EKC][:, kc % EKC, :, :],
                        sc[:],
                        mybir.ActivationFunctionType.Exp,
                        scale=0.125,
                    )

            def emit_pv(qb, hp, es_tiles, attn, vT):
                pv = [psum.tile([65, QB], F32, name=f"pv{hi}", bufs=1)
                      for hi in range(2)]
                for kc in range(KC):
                    for hi in range(2):
                        nc.tensor.matmul(
                            pv[hi][:],
                            vT[:, hp, kc, hi * 65:(hi + 1) * 65],
                            es_tiles[kc // EKC][:, kc % EKC, hi, :],
                            start=(kc == 0),
                            stop=(kc == KC - 1),
                        )
                for hi in range(2):
                    inv = small.tile([1, QB], F32, name="inv")
                    nc.vector.reciprocal(inv[:], pv[hi][64:65, :])
                    inv_b = small.tile([D, QB], F32, name="invb")
                    nc.gpsimd.partition_broadcast(inv_b[:], inv[:], channels=D)
                    nc.vector.tensor_tensor(
                        out=attn[hi * D:(hi + 1) * D, hp, :],
                        in0=pv[hi][0:64, :],
                        in1=inv_b[:],
                        op=mybir.AluOpType.mult,
                    )

            def emit_outproj(qb, attn):
                for tci in range(QB // P):
                    tc_g = (qb * QB + tci * P) // P
                    out_sb = outp.tile([P, E], BF16, name="osb")
                    for eb in range(E // 512):
                        op = psum.tile([P, 512], F32, name="work", bufs=2)
                        for hp in range(2):
                            nc.tensor.matmul(
                                op[:],
                                attn[:, hp, tci * P:(tci + 1) * P],
                                w_outT_sb[:, hp, eb * 512:(eb + 1) * 512],
                                start=(hp == 0),
                                stop=(hp == 1),
                            )
                        nc.vector.tensor_copy(
                            out_sb[:, eb * 512:(eb + 1) * 512], op[:]
                        )
                    nc.sync.dma_start(out_d[tc_g], out_sb[:])

            def emit_qkv_head(vT):
                # next repeat's QKV head: first half of tokens + first
                # phase's scores for kc 0-7 interleaved, then second half
                nc.vector.memset(vT[:, :, :, 64:65], 1.0)
                nc.vector.memset(vT[:, :, :, 129:130], 1.0)
                emit_tb(0, vT)
                emit_tb(1, vT)

            # ---- prologue: repeat 0's QKV + first phase's scores ----
            phases = [(qb, hp) for qb in range(NQB) for hp in range(2)]
            vT_cur = vt_p.tile([P, 2, KC, 130], BF16, name="vT")
            emit_qkv_head(vT_cur)
            es_next = {}
            emit_scores(0, 0, range(0, 8), es_next)
            emit_tb(2, vT_cur)
            emit_tb(3, vT_cur)
            emit_scores(0, 0, range(8, 16), es_next)

            for r in range(repeat):
                last = r == repeat - 1
                vT_nxt = None
                attn = None
                for i, (qb, hp) in enumerate(phases):
                    es_cur = es_next
                    es_next = {}
                    if hp == 0:
                        attn = attn_p.tile([P, 2, QB], BF16, name="attn")
                    if i < 6:
                        nqb, nhp = phases[i + 1]
                        emit_scores(nqb, nhp, range(KC), es_next)
                        emit_pv(qb, hp, es_cur, attn, vT_cur)
                    elif i == 6:
                        nqb, nhp = phases[7]
                        emit_scores(nqb, nhp, range(KC), es_next)
                        emit_pv(qb, hp, es_cur, attn, vT_cur)
                        if not last:
                            vT_nxt = vt_p.tile([P, 2, KC, 130], BF16, name="vT")
                            emit_qkv_head(vT_nxt)
                            emit_scores(0, 0, range(0, 8), es_next2 := {})
                            emit_tb(2, vT_nxt)
                            emit_tb(3, vT_nxt)
                    else:
                        emit_pv(qb, hp, es_cur, attn, vT_cur)
                        emit_outproj(qb, attn)
                        if not last:
                            emit_scores(0, 0, range(8, 16), es_next2)
                            es_next = es_next2
                            vT_cur = vT_nxt
                    if hp == 1 and i < 7:
                        emit_outproj(qb, attn)

    nc.compile()
    return nc


def _prep_inputs(x, w_in, b_in, w_out):
    x = np.ascontiguousarray(np.asarray(x, dtype=np.float32))
    w_in = np.asarray(w_in, dtype=np.float32)
    b_in = np.asarray(b_in, dtype=np.float32)
    w_out = np.asarray(w_out, dtype=np.float32)

    xT_b = []
    for b in range(B):
        xT_b.append(np.ascontiguousarray(
            x[b].T.reshape(EC, P, S).transpose(1, 0, 2)
        ).astype(ml_dtypes.bfloat16))          # [128, EC, S]

    in_maps = []
    for c in range(NCORES):
        b = c // 4
        hg = c % 4
        h0 = hg * HC                            # first head of this core
        # 6 feature chunks: (hp, j) -> rows of w_in
        rows = []
        brows = []
        for hp in range(2):
            for j in range(3):                  # q, k, v
                for hi in range(2):
                    h = h0 + hp * 2 + hi
                    r0 = j * E + h * D
                    rows.append(w_in[r0:r0 + D])
                    brows.append(b_in[r0:r0 + D])
        rows = np.concatenate(rows)             # [768, 1024]
        w_inT_c = np.ascontiguousarray(
            rows.T.reshape(EC, P, NCH * P).transpose(1, 0, 2)
        ).astype(ml_dtypes.bfloat16)            # [128, EC, 768]
        b_c = np.ascontiguousarray(
            np.concatenate(brows).reshape(NCH, P).T
        )                                       # [128, 6]
        # w_outT [128 = hi*64+d, hp, E]
        w_outT_c = np.empty((P, 2, E), dtype=np.float32)
        for hp in range(2):
            for hi in range(2):
                h = h0 + hp * 2 + hi
                w_outT_c[hi * D:(hi + 1) * D, hp] = w_out[:, h * D:(h + 1) * D].T
        w_outT_c = w_outT_c.astype(ml_dtypes.bfloat16)
        in_maps.append({
            "xT": xT_b[b],
            "w_inT": w_inT_c,
            "b_in": b_c,
            "w_outT": w_outT_c,
        })
    return in_maps


def kernel(x, w_in, b_in, w_out, b_out, _trace=False):
    global _COMPILED
    if _COMPILED is None:
        _COMPILED = build()
    nc = _COMPILED

    in_maps = _prep_inputs(x, w_in, b_in, w_out)
    res = run_bass_kernel_spmd(
        nc, in_maps, core_ids=list(range(NCORES)), trace=_trace
    )
    out = np.zeros((B, S, E), dtype=np.float32)
    for c in range(NCORES):
        b = c // 4
        out[b] += res.results[c]["out"].astype(np.float32).reshape(S, E)
    out += np.asarray(b_out, dtype=np.float32)
    if _trace:
        return out, res
    return out


# revision 5
# speedup vs baseline: 2.4214x; 1.7368x over previous
"""Multi-head attention (B=2, S=2048, E=1024, H=16) on 8 TRN2 NeuronCores.

Sharding: batch x head-quarter. Core c handles batch c//4 and heads
{4*(c%4) .. 4*(c%4)+3} end to end (QKV slice, attention, row-parallel slice
of out_proj over its 4 heads' 256 contraction dims). Each core returns a
bf16 partial [2048, 1024]; the host sums groups of 4 in fp32 and adds b_out.

The device program is software-pipelined around the ACT engine (the exp of
134M/8 attention weights is the largest single-engine load): the kernel is
split into 8 phases (4 q-blocks of 512 x 2 head-pairs); scores for phase
i+1 are emitted before PV of phase i so ACT always has exp work queued, and
the next repeat's QKV runs inside the current repeat's tail (qk tiles are
read-complete by then; vT is double-buffered).

Per-phase structure:
  scores: psum [128 k, 2 hi, 512 q] per kc; the head pair is row-packed
          (tile_position=(64*hi, 0)) so both heads' score matmuls run
          concurrently on the PE; one exp per (kc, hi) -> es bf16.
  PV:     psum [65, 512] per hi = [v | 1].T @ es accumulated over 16 kc
          (row 64 = softmax sums; es tiles are consumed kc-major and free
          early); normalize via reciprocal + gpsimd partition_broadcast +
          DVE multiply into attn [128, hp, 512] (hi stacked across
          partition halves).
  out_proj (per q-block): psum [128 t, 512 e] accumulates 2 K=128
          head-pair matmuls; evict bf16 + DMA partial out.
"""
import sys

sys.path.insert(0, "/opt/trn_rl_repo")
import numpy as np
import ml_dtypes
import concourse.bass as bass
import concourse.mybir as mybir
import concourse.tile as tile
from concourse import bacc
from concourse.bass_utils import run_bass_kernel_spmd
from concourse.masks import make_identity

P = 128
B = 2
S = 2048          # tokens per core (= one batch)
E = 1024
H = 16
D = 64            # head dim
HC = 4            # heads per core
NCORES = 8
EC = E // P       # 8 contraction chunks for QKV
NCH = 3 * HC * D // P   # 6 feature chunks (q|k|v x 2 head pairs)
QB = 512          # q block (phase) size
NQB = S // QB     # 4 q blocks
KC = S // P       # 16 k chunks
TB = 512          # token block for streaming xT
NTB = S // TB     # 4
EKC = 2           # kc chunks grouped per es tile

F32 = mybir.dt.float32
BF16 = mybir.dt.bfloat16

_COMPILED = None


def build(repeat=1):
    nc = bacc.Bacc(None, target_bir_lowering=False)
    xT_d = nc.dram_tensor("xT", [P, EC, S], BF16, kind="ExternalInput")
    w_inT_d = nc.dram_tensor("w_inT", [P, EC, NCH * P], BF16, kind="ExternalInput")
    b_in_d = nc.dram_tensor("b_in", [P, NCH], F32, kind="ExternalInput")
    w_outT_d = nc.dram_tensor("w_outT", [P, 2, E], BF16, kind="ExternalInput")
    out_d = nc.dram_tensor("out", [S // P, P, E], BF16, kind="ExternalOutput")

    with tile.TileContext(nc) as tc:
        with (
            tc.tile_pool(name="const", bufs=1) as const,
            tc.tile_pool(name="qk_p", bufs=1) as qk_p,
            tc.tile_pool(name="vt_p", bufs=2) as vt_p,
            tc.tile_pool(name="attn_p", bufs=2) as attn_p,
            tc.tile_pool(name="outp", bufs=3) as outp,
            tc.tile_pool(name="small", bufs=2) as small,
            tc.tile_pool(name="qkv_in", bufs=2) as qkv_in,
            tc.tile_pool(name="es_pool", bufs=14) as es_pool,
            tc.tile_pool(name="psum", bufs=1, space="PSUM") as psum,
        ):
            identity = const.tile([P, P], BF16)
            make_identity(nc, identity)
            b_in_sb = const.tile([P, NCH], F32)
            nc.sync.dma_start(b_in_sb[:], b_in_d[:])
            w_outT_sb = const.tile([P, 2, E], BF16)
            nc.sync.dma_start(w_outT_sb[:], w_outT_d[:])
            w_inT_sb = const.tile([P, EC, NCH * P], BF16)
            nc.sync.dma_start(w_inT_sb[:], w_inT_d[:])

            # q,k feature-major, one tile per (hp, q|k): [128 = hi*64+d, token]
            qk = [[qk_p.tile([P, S], BF16, name=f"qk{hp}{j}")
                   for j in range(2)] for hp in range(2)]

            def emit_tb(tbi, vT):
                xt = qkv_in.tile([P, EC, TB], BF16, name="xt")
                nc.sync.dma_start(xt[:], xT_d[:, :, tbi * TB:(tbi + 1) * TB])
                for hp in range(2):
                    v_sb = qkv_in.tile([P, TB], BF16, name="vsb")
                    for j in range(3):          # q, k, v
                        ch = hp * 3 + j
                        acc = psum.tile([P, TB], F32, name="work", bufs=2)
                        for ec in range(EC):
                            nc.tensor.matmul(
                                acc[:],
                                w_inT_sb[:, ec, ch * P:(ch + 1) * P],
                                xt[:, ec, :],
                                start=(ec == 0),
                                stop=(ec == EC - 1),
                            )
                        nc.vector.tensor_scalar(
                            out=(qk[hp][j][:, tbi * TB:(tbi + 1) * TB]
                                 if j < 2 else v_sb[:]),
                            in0=acc[:],
                            scalar1=b_in_sb[:, ch:ch + 1],
                            scalar2=None,
                            op0=mybir.AluOpType.add,
                        )
                    for kci in range(TB // P):
                        kc = tbi * (TB // P) + kci
                        tp = psum.tile([P, P], BF16, name="work", bufs=2)
                        nc.tensor.transpose(
                            tp[:], v_sb[:, kci * P:(kci + 1) * P], identity[:]
                        )
                        nc.vector.tensor_copy(vT[:, hp, kc, 0:64], tp[:, 0:64])
                        nc.vector.tensor_copy(vT[:, hp, kc, 65:129], tp[:, 64:128])

            def emit_scores(qb, hp, kcs, es_tiles):
                q0 = qb * QB
                sc = None
                for kc in kcs:
                    if kc % EKC == 0:
                        sc = psum.tile([P, EKC, 2, QB], F32, name="sc", bufs=1)
                    for hi in range(2):
                        nc.tensor.matmul(
                            sc[:, kc % EKC, hi, :],
                            qk[hp][1][hi * D:(hi + 1) * D, kc * P:(kc + 1) * P],
                            qk[hp][0][hi * D:(hi + 1) * D, q0:q0 + QB],
                            start=True, stop=True,
                            tile_position=(hi * D, 0),
                        )
                    if kc % EKC == EKC - 1:
                        es_tiles[kc // EKC] = es_pool.tile(
                            [P, EKC, 2, QB], BF16, name="es"
                        )
                        # one exp over both kc's and both heads: N=2048
                        nc.scalar.activation(
                            es_tiles[kc // EKC][:],
                            sc[:],
                            mybir.ActivationFunctionType.Exp,
                            scale=0.125,
                        )

            def emit_pv(qb, hp, es_tiles, attn, vT):
                pv = [psum.tile([65, QB], F32, name=f"pv{hi}", bufs=1)
                      for hi in range(2)]
                for kc in range(KC):
                    for hi in range(2):
                        nc.tensor.matmul(
                            pv[hi][:],
                            vT[:, hp, kc, hi * 65:(hi + 1) * 65],
                            es_tiles[kc // EKC][:, kc % EKC, hi, :],
                            start=(kc == 0),
                            stop=(kc == KC - 1),
                        )
                for hi in range(2):
                    inv = small.tile([1, QB], F32, name="inv")
                    nc.vector.reciprocal(inv[:], pv[hi][64:65, :])
                    inv_b = small.tile([D, QB], F32, name="invb")
                    nc.gpsimd.partition_broadcast(inv_b[:], inv[:], channels=D)
                    nc.vector.tensor_tensor(
                        out=attn[hi * D:(hi + 1) * D, hp, :],
                        in0=pv[hi][0:64, :],
                        in1=inv_b[:],
                        op=mybir.AluOpType.mult,
                    )

            def emit_outproj(qb, attn):
                for tci in range(QB // P):
                    tc_g = (qb * QB + tci * P) // P
                    out_sb = outp.tile([P, E], BF16, name="osb")
                    for eb in range(E // 512):
                        op = psum.tile([P, 512], F32, name="work", bufs=2)
                        for hp in range(2):
                            nc.tensor.matmul(
                                op[:],
                                attn[:, hp, tci * P:(tci + 1) * P],
                                w_outT_sb[:, hp, eb * 512:(eb + 1) * 512],
                                start=(hp == 0),
                                stop=(hp == 1),
                            )
                        nc.vector.tensor_copy(
                            out_sb[:, eb * 512:(eb + 1) * 512], op[:]
                        )
                    nc.sync.dma_start(out_d[tc_g], out_sb[:])

            def emit_qkv_head(vT):
                # next repeat's QKV head: first half of tokens + first
                # phase's scores for kc 0-7 interleaved, then second half
                nc.vector.memset(vT[:, :, :, 64:65], 1.0)
                nc.vector.memset(vT[:, :, :, 129:130], 1.0)
                emit_tb(0, vT)
                emit_tb(1, vT)

            # ---- prologue: repeat 0's QKV + first phase's scores ----
            phases = [(qb, hp) for qb in range(NQB) for hp in range(2)]
            vT_cur = vt_p.tile([P, 2, KC, 130], BF16, name="vT")
            emit_qkv_head(vT_cur)
            es_next = {}
            emit_scores(0, 0, range(0, 8), es_next)
            emit_tb(2, vT_cur)
            emit_tb(3, vT_cur)
            emit_scores(0, 0, range(8, 16), es_next)

            for r in range(repeat):
                last = r == repeat - 1
                vT_nxt = None
                attn = None
                for i, (qb, hp) in enumerate(phases):
                    es_cur = es_next
                    es_next = {}
                    if hp == 0:
                        attn = attn_p.tile([P, 2, QB], BF16, name="attn")
                    if i < 6:
                        nqb, nhp = phases[i + 1]
                        emit_scores(nqb, nhp, range(KC), es_next)
                        emit_pv(qb, hp, es_cur, attn, vT_cur)
                    elif i == 6:
                        nqb, nhp = phases[7]
                        emit_scores(nqb, nhp, range(KC), es_next)
                        emit_pv(qb, hp, es_cur, attn, vT_cur)
                        if not last:
                            vT_nxt = vt_p.tile([P, 2, KC, 130], BF16, name="vT")
                            emit_qkv_head(vT_nxt)
                            emit_scores(0, 0, range(0, 8), es_next2 := {})
                            emit_tb(2, vT_nxt)
                            emit_tb(3, vT_nxt)
                    else:
                        emit_pv(qb, hp, es_cur, attn, vT_cur)
                        emit_outproj(qb, attn)
                        if not last:
                            emit_scores(0, 0, range(8, 16), es_next2)
                            es_next = es_next2
                            vT_cur = vT_nxt
                    if hp == 1 and i < 7:
                        emit_outproj(qb, attn)

    nc.compile()
    return nc


def _prep_inputs(x, w_in, b_in, w_out):
    x = np.ascontiguousarray(np.asarray(x, dtype=np.float32))
    w_in = np.asarray(w_in, dtype=np.float32)
    b_in = np.asarray(b_in, dtype=np.float32)
    w_out = np.asarray(w_out, dtype=np.float32)

    xT_b = []
    for b in range(B):
        xT_b.append(np.ascontiguousarray(
            x[b].T.reshape(EC, P, S).transpose(1, 0, 2)
        ).astype(ml_dtypes.bfloat16))          # [128, EC, S]

    in_maps = []
    for c in range(NCORES):
        b = c // 4
        hg = c % 4
        h0 = hg * HC                            # first head of this core
        # 6 feature chunks: (hp, j) -> rows of w_in
        rows = []
        brows = []
        for hp in range(2):
            for j in range(3):                  # q, k, v
                for hi in range(2):
                    h = h0 + hp * 2 + hi
                    r0 = j * E + h * D
                    rows.append(w_in[r0:r0 + D])
                    brows.append(b_in[r0:r0 + D])
        rows = np.concatenate(rows)             # [768, 1024]
        w_inT_c = np.ascontiguousarray(
            rows.T.reshape(EC, P, NCH * P).transpose(1, 0, 2)
        ).astype(ml_dtypes.bfloat16)            # [128, EC, 768]
        b_c = np.ascontiguousarray(
            np.concatenate(brows).reshape(NCH, P).T
        )                                       # [128, 6]
        # w_outT [128 = hi*64+d, hp, E]
        w_outT_c = np.empty((P, 2, E), dtype=np.float32)
        for hp in range(2):
            for hi in range(2):
                h = h0 + hp * 2 + hi
                w_outT_c[hi * D:(hi + 1) * D, hp] = w_out[:, h * D:(h + 1) * D].T
        w_outT_c = w_outT_c.astype(ml_dtypes.bfloat16)
        in_maps.append({
            "xT": xT_b[b],
            "w_inT": w_inT_c,
            "b_in": b_c,
            "w_outT": w_outT_c,
        })
    return in_maps


def kernel(x, w_in, b_in, w_out, b_out, _trace=False):
    global _COMPILED
    if _COMPILED is None:
        _COMPILED = build()
    nc = _COMPILED

    in_maps = _prep_inputs(x, w_in, b_in, w_out)
    res = run_bass_kernel_spmd(
        nc, in_maps, core_ids=list(range(NCORES)), trace=_trace
    )
    out = np.zeros((B, S, E), dtype=np.float32)
    for c in range(NCORES):
        b = c // 4
        out[b] += res.results[c]["out"].astype(np.float32).reshape(S, E)
    out += np.asarray(b_out, dtype=np.float32)
    if _trace:
        return out, res
    return out


# revision 8
# speedup vs baseline: 5.6211x; 2.3214x over previous
"""Multi-head attention (B=2, S=2048, E=1024, H=16) on 8 TRN2 NeuronCores.

Sharding: tensor-parallel on heads - core c computes heads {2c, 2c+1} end to
end (QKV projection slice, attention, and the row-parallel slice of out_proj
over its 2 heads' 128 contraction dims), and returns a partial [4096, 1024]
fp32 output; the host sums the 8 partials and adds b_out.

The device program is software-pipelined around the PE (the matmul mix is
the largest single-engine load; the exp stream on ACT is second): the work
is split into 8 phases (2 batches x 4 q-blocks of 512); scores for phase
i+1 are emitted before PV of phase i so ACT always has exp work queued, and
the next repeat's QKV projection runs inside the current repeat's tail (qk
tiles are read-complete by then; vT is double-buffered).

Per-phase structure:
  scores: psum [128 k, 2 hi, 512 q] per kc; the head pair is row-packed
          (tile_position=(64*hi, 0)) so both heads' score matmuls run
          concurrently on the PE; one exp per kc (N=1024) -> es bf16.
  PV:     psum [65, 512] per hi = [v | 1].T @ es accumulated over 16 kc
          (row 64 = softmax sums; es tiles are consumed kc-major and free
          early); normalize via reciprocal + gpsimd partition_broadcast +
          DVE multiply into attn [128, 512] (hi stacked across partition
          halves).
  out_proj (per phase): psum [128 t, 512 e] via a single K=128 head-stacked
          matmul per (t-chunk, e-block); evict fp32 + DMA partial out.
"""
import sys

sys.path.insert(0, "/opt/trn_rl_repo")
import numpy as np
import ml_dtypes
import concourse.bass as bass
import concourse.mybir as mybir
import concourse.tile as tile
from concourse import bacc
from concourse.bass_utils import run_bass_kernel_spmd
from concourse.masks import make_identity

P = 128
B = 2
S = 2048          # sequence length per batch
T = B * S         # 4096 global tokens
E = 1024
H = 16
D = 64            # head dim
NCORES = 8
EC = E // P       # 8 contraction chunks for QKV
NCH = 3           # feature chunks: q | k | v of the head pair
QB = 512          # q block (phase) size
NQB = S // QB     # 4 q blocks per batch
KC = S // P       # 16 k chunks per batch
TB = 512          # token block for streaming xT
NTB = T // TB     # 8
EKC = 2           # kc chunks grouped per es tile

F32 = mybir.dt.float32
BF16 = mybir.dt.bfloat16

_COMPILED = None


def build(repeat=1):
    nc = bacc.Bacc(None, target_bir_lowering=False)
    xT_d = nc.dram_tensor("xT", [P, EC, T], BF16, kind="ExternalInput")
    w_inT_d = nc.dram_tensor("w_inT", [P, EC, NCH * P], BF16, kind="ExternalInput")
    b_in_d = nc.dram_tensor("b_in", [P, NCH], F32, kind="ExternalInput")
    w_outT_d = nc.dram_tensor("w_outT", [P, E], BF16, kind="ExternalInput")
    out_d = nc.dram_tensor("out", [T // P, P, E], F32, kind="ExternalOutput")

    with tile.TileContext(nc) as tc:
        with (
            tc.tile_pool(name="const", bufs=1) as const,
            tc.tile_pool(name="qk_p", bufs=1) as qk_p,
            tc.tile_pool(name="vt_p", bufs=2) as vt_p,
            tc.tile_pool(name="attn_p", bufs=2) as attn_p,
            tc.tile_pool(name="outp", bufs=3) as outp,
            tc.tile_pool(name="small", bufs=2) as small,
            tc.tile_pool(name="qkv_in", bufs=2) as qkv_in,
            tc.tile_pool(name="es_pool", bufs=14) as es_pool,
            tc.tile_pool(name="psum", bufs=1, space="PSUM") as psum,
        ):
            identity = const.tile([P, P], BF16)
            make_identity(nc, identity)
            b_in_sb = const.tile([P, NCH], F32)
            nc.sync.dma_start(b_in_sb[:], b_in_d[:])
            w_outT_sb = const.tile([P, E], BF16)
            nc.sync.dma_start(w_outT_sb[:], w_outT_d[:])
            w_inT_sb = const.tile([P, EC, NCH * P], BF16)
            nc.sync.dma_start(w_inT_sb[:], w_inT_d[:])

            # q,k feature-major, one tile per (q|k): [128 = hi*64+d, token]
            qk = [qk_p.tile([P, T], BF16, name=f"qk{j}") for j in range(2)]

            def emit_tb(tbi, vT):
                xt = qkv_in.tile([P, EC, TB], BF16, name="xt")
                nc.sync.dma_start(xt[:], xT_d[:, :, tbi * TB:(tbi + 1) * TB])
                v_sb = qkv_in.tile([P, TB], BF16, name="vsb")
                for j in range(3):          # q, k, v
                    acc = psum.tile([P, TB], F32, name="work", bufs=2)
                    for ec in range(EC):
                        nc.tensor.matmul(
                            acc[:],
                            w_inT_sb[:, ec, j * P:(j + 1) * P],
                            xt[:, ec, :],
                            start=(ec == 0),
                            stop=(ec == EC - 1),
                        )
                    nc.vector.tensor_scalar(
                        out=(qk[j][:, tbi * TB:(tbi + 1) * TB]
                             if j < 2 else v_sb[:]),
                        in0=acc[:],
                        scalar1=b_in_sb[:, j:j + 1],
                        scalar2=None,
                        op0=mybir.AluOpType.add,
                    )
                for kci in range(TB // P):
                    kcg = tbi * (TB // P) + kci     # global k chunk 0..31
                    tp = psum.tile([P, P], BF16, name="work", bufs=2)
                    nc.tensor.transpose(
                        tp[:], v_sb[:, kci * P:(kci + 1) * P], identity[:]
                    )
                    nc.vector.tensor_copy(
                        vT[:, kcg // KC, kcg % KC, 0:64], tp[:, 0:64])
                    nc.vector.tensor_copy(
                        vT[:, kcg // KC, kcg % KC, 65:129], tp[:, 64:128])

            def emit_scores(b, qb, kcs, es_tiles):
                q0 = b * S + qb * QB
                k0 = b * S
                for kc in kcs:
                    sc = psum.tile([P, 2, QB], F32, name="sc", bufs=2)
                    for hi in range(2):
                        nc.tensor.matmul(
                            sc[:, hi, :],
                            qk[1][hi * D:(hi + 1) * D,
                                  k0 + kc * P:k0 + (kc + 1) * P],
                            qk[0][hi * D:(hi + 1) * D, q0:q0 + QB],
                            start=True, stop=True,
                            tile_position=(hi * D, 0),
                        )
                    if kc % EKC == 0:
                        es_tiles[kc // EKC] = es_pool.tile(
                            [P, EKC, 2, QB], BF16, name="es"
                        )
                    nc.scalar.activation(
                        es_tiles[kc // EKC][:, kc % EKC, :, :],
                        sc[:, :, :],
                        mybir.ActivationFunctionType.Exp,
                        scale=0.125,
                    )

            def emit_pv(b, qb, es_tiles, attn, vT):
                pv = [psum.tile([65, QB], F32, name=f"pv{hi}", bufs=1)
                      for hi in range(2)]
                for kc in range(KC):
                    for hi in range(2):
                        nc.tensor.matmul(
                            pv[hi][:],
                            vT[:, b, kc, hi * 65:(hi + 1) * 65],
                            es_tiles[kc // EKC][:, kc % EKC, hi, :],
                            start=(kc == 0),
                            stop=(kc == KC - 1),
                        )
                for hi in range(2):
                    inv = small.tile([1, QB], F32, name="inv")
                    nc.vector.reciprocal(inv[:], pv[hi][64:65, :])
                    inv_b = small.tile([D, QB], F32, name="invb")
                    nc.gpsimd.partition_broadcast(inv_b[:], inv[:], channels=D)
                    nc.vector.tensor_tensor(
                        out=attn[hi * D:(hi + 1) * D, :],
                        in0=pv[hi][0:64, :],
                        in1=inv_b[:],
                        op=mybir.AluOpType.mult,
                    )

            def emit_outproj(b, qb, attn):
                for tci in range(QB // P):
                    tc_g = (b * S + qb * QB + tci * P) // P
                    out_sb = outp.tile([P, E], F32, name="osb")
                    for eb in range(E // 512):
                        op = psum.tile([P, 512], F32, name="work", bufs=2)
                        nc.tensor.matmul(
                            op[:],
                            attn[:, tci * P:(tci + 1) * P],
                            w_outT_sb[:, eb * 512:(eb + 1) * 512],
                            start=True, stop=True,
                        )
                        nc.vector.tensor_copy(
                            out_sb[:, eb * 512:(eb + 1) * 512], op[:]
                        )
                    nc.sync.dma_start(out_d[tc_g], out_sb[:])

            def emit_qkv_head(vT):
                nc.vector.memset(vT[:, :, :, 64:65], 1.0)
                nc.vector.memset(vT[:, :, :, 129:130], 1.0)
                emit_tb(0, vT)
                emit_tb(1, vT)

            def emit_qkv_tail(vT, es2):
                emit_scores(0, 0, range(0, 8), es2)
                for tbi in range(2, NTB):
                    emit_tb(tbi, vT)

            # ---- prologue: repeat 0's QKV + first phase's scores ----
            phases = [(b, qb) for b in range(B) for qb in range(NQB)]
            vT_cur = vt_p.tile([P, B, KC, 130], BF16, name="vT")
            emit_qkv_head(vT_cur)
            es_next = {}
            emit_qkv_tail(vT_cur, es_next)
            emit_scores(0, 0, range(8, 16), es_next)

            for r in range(repeat):
                last = r == repeat - 1
                vT_nxt = None
                es_next2 = None
                for i, (b, qb) in enumerate(phases):
                    es_cur = es_next
                    es_next = {}
                    attn = attn_p.tile([P, QB], BF16, name="attn")
                    if i < 7:
                        nb, nqb = phases[i + 1]
                        emit_scores(nb, nqb, range(KC), es_next)
                        emit_pv(b, qb, es_cur, attn, vT_cur)
                        if i == 6 and not last:
                            vT_nxt = vt_p.tile([P, B, KC, 130], BF16, name="vT")
                            emit_qkv_head(vT_nxt)
                            emit_qkv_tail(vT_nxt, es_next2 := {})
                    else:
                        emit_pv(b, qb, es_cur, attn, vT_cur)
                        if not last:
                            emit_scores(0, 0, range(8, 16), es_next2)
                            es_next = es_next2
                            vT_cur = vT_nxt
                    emit_outproj(b, qb, attn)

    nc.compile()
    return nc


def _prep_inputs(x, w_in, b_in, w_out):
    x = np.ascontiguousarray(np.asarray(x, dtype=np.float32))
    w_in = np.asarray(w_in, dtype=np.float32)
    b_in = np.asarray(b_in, dtype=np.float32)
    w_out = np.asarray(w_out, dtype=np.float32)

    xT = np.ascontiguousarray(
        x.reshape(T, E).T.reshape(EC, P, T).transpose(1, 0, 2)
    ).astype(ml_dtypes.bfloat16)                # [128, EC, T]

    in_maps = []
    for c in range(NCORES):
        h0 = 2 * c
        rows = []
        brows = []
        for j in range(3):                      # q, k, v
            for hi in range(2):
                r0 = j * E + (h0 + hi) * D
                rows.append(w_in[r0:r0 + D])
                brows.append(b_in[r0:r0 + D])
        rows = np.concatenate(rows)             # [384, 1024]
        w_inT_c = np.ascontiguousarray(
            rows.T.reshape(EC, P, NCH * P).transpose(1, 0, 2)
        ).astype(ml_dtypes.bfloat16)            # [128, EC, 384]
        b_c = np.ascontiguousarray(
            np.concatenate(brows).reshape(NCH, P).T
        )                                       # [128, 3]
        # w_outT [128 = hi*64+d, E]
        w_outT_c = np.empty((P, E), dtype=np.float32)
        for hi in range(2):
            h = h0 + hi
            w_outT_c[hi * D:(hi + 1) * D] = w_out[:, h * D:(h + 1) * D].T
        w_outT_c = w_outT_c.astype(ml_dtypes.bfloat16)
        in_maps.append({
            "xT": xT,
            "w_inT": w_inT_c,
            "b_in": b_c,
            "w_outT": w_outT_c,
        })
    return in_maps


def kernel(x, w_in, b_in, w_out, b_out, _trace=False):
    global _COMPILED
    if _COMPILED is None:
        _COMPILED = build()
    nc = _COMPILED

    in_maps = _prep_inputs(x, w_in, b_in, w_out)
    res = run_bass_kernel_spmd(
        nc, in_maps, core_ids=list(range(NCORES)), trace=_trace
    )
    partial = np.zeros((T // P, P, E), dtype=np.float32)
    for c in range(NCORES):
        partial += res.results[c]["out"]
    out = partial.reshape(T, E) + np.asarray(b_out, dtype=np.float32)
    out = out.reshape(B, S, E)
    if _trace:
        return out, res
    return out


# revision 10
# speedup vs baseline: 12.7297x; 2.2647x over previous
"""Multi-head attention (B=2, S=2048, E=1024, H=16) on 8 TRN2 NeuronCores.

Sharding: tensor-parallel on heads - core c computes heads {2c, 2c+1} end to
end (QKV projection slice, attention, and the row-parallel slice of out_proj
over its 2 heads' 128 contraction dims), and returns a partial [4096, 1024]
fp32 output; the host sums the 8 partials and adds b_out.

The device program is software-pipelined around the PE (the matmul mix is
the largest single-engine load; the exp stream on ACT is second): the work
is split into 8 phases (2 batches x 4 q-blocks of 512); scores for phase
i+1 are emitted before PV of phase i so ACT always has exp work queued, and
the next repeat's QKV projection runs inside the current repeat's tail (qk
tiles are read-complete by then; vT is double-buffered).

Per-phase structure:
  scores: psum [128 k, 2 hi, 512 q] per kc; the head pair is row-packed
          (tile_position=(64*hi, 0)) so both heads' score matmuls run
          concurrently on the PE; one exp per kc (N=1024) -> es bf16.
  PV:     psum [65, 512] per hi = [v | 1].T @ es accumulated over 16 kc
          (row 64 = softmax sums; es tiles are consumed kc-major and free
          early); normalize via reciprocal + gpsimd partition_broadcast +
          DVE multiply into attn [128, 512] (hi stacked across partition
          halves).
  out_proj (per phase): psum [128 t, 512 e] via a single K=128 head-stacked
          matmul per (t-chunk, e-block); evict fp32 + DMA partial out.
"""
import sys

sys.path.insert(0, "/opt/trn_rl_repo")
import numpy as np
import ml_dtypes
import concourse.bass as bass
import concourse.mybir as mybir
import concourse.tile as tile
from concourse import bacc
from concourse.bass_utils import run_bass_kernel_spmd
from concourse.masks import make_identity

P = 128
B = 2
S = 2048          # sequence length per batch
T = B * S         # 4096 global tokens
E = 1024
H = 16
D = 64            # head dim
NCORES = 8
EC = E // P       # 8 contraction chunks for QKV
NCH = 3           # feature chunks: q | k | v of the head pair
QB = 512          # q block (phase) size
NQB = S // QB     # 4 q blocks per batch
KC = S // P       # 16 k chunks per batch
TB = 512          # token block for streaming xT
NTB = T // TB     # 8
EKC = 2           # kc chunks grouped per es tile

F32 = mybir.dt.float32
BF16 = mybir.dt.bfloat16

_COMPILED = None


def build(repeat=1):
    nc = bacc.Bacc(None, target_bir_lowering=False)
    xT_d = nc.dram_tensor("xT", [P, EC, T], BF16, kind="ExternalInput")
    w_inT_d = nc.dram_tensor("w_inT", [P, EC, NCH * P], BF16, kind="ExternalInput")
    b_in_d = nc.dram_tensor("b_in", [P, NCH], F32, kind="ExternalInput")
    w_outT_d = nc.dram_tensor("w_outT", [P, E], BF16, kind="ExternalInput")
    out_d = nc.dram_tensor("out", [T // P, P, E], F32, kind="ExternalOutput")

    with tile.TileContext(nc) as tc:
        with (
            tc.tile_pool(name="const", bufs=1) as const,
            tc.tile_pool(name="qk_p", bufs=1) as qk_p,
            tc.tile_pool(name="vt_p", bufs=2) as vt_p,
            tc.tile_pool(name="attn_p", bufs=2) as attn_p,
            tc.tile_pool(name="outp", bufs=3) as outp,
            tc.tile_pool(name="small", bufs=2) as small,
            tc.tile_pool(name="qkv_in", bufs=2) as qkv_in,
            tc.tile_pool(name="es_pool", bufs=14) as es_pool,
            tc.tile_pool(name="psum", bufs=1, space="PSUM") as psum,
        ):
            identity = const.tile([P, P], BF16)
            make_identity(nc, identity)
            b_in_sb = const.tile([P, NCH], F32)
            nc.sync.dma_start(b_in_sb[:], b_in_d[:])
            w_outT_sb = const.tile([P, E], BF16)
            nc.sync.dma_start(w_outT_sb[:], w_outT_d[:])
            w_inT_sb = const.tile([P, EC, NCH * P], BF16)
            nc.sync.dma_start(w_inT_sb[:], w_inT_d[:])

            # q,k feature-major, one tile per (q|k): [128 = hi*64+d, token]
            qk = [qk_p.tile([P, T], BF16, name=f"qk{j}") for j in range(2)]

            def emit_tb(tbi, vT):
                xt = qkv_in.tile([P, EC, TB], BF16, name="xt")
                nc.sync.dma_start(xt[:], xT_d[:, :, tbi * TB:(tbi + 1) * TB])
                v_sb = qkv_in.tile([P, TB], BF16, name="vsb")
                for j in range(3):          # q, k, v
                    acc = psum.tile([P, TB], F32, name="work", bufs=2)
                    for ec in range(EC):
                        nc.tensor.matmul(
                            acc[:],
                            w_inT_sb[:, ec, j * P:(j + 1) * P],
                            xt[:, ec, :],
                            start=(ec == 0),
                            stop=(ec == EC - 1),
                        )
                    nc.vector.tensor_scalar(
                        out=(qk[j][:, tbi * TB:(tbi + 1) * TB]
                             if j < 2 else v_sb[:]),
                        in0=acc[:],
                        scalar1=b_in_sb[:, j:j + 1],
                        scalar2=None,
                        op0=mybir.AluOpType.add,
                    )
                for kci in range(TB // P):
                    kcg = tbi * (TB // P) + kci     # global k chunk 0..31
                    tp = psum.tile([P, P], BF16, name="work", bufs=2)
                    nc.tensor.transpose(
                        tp[:], v_sb[:, kci * P:(kci + 1) * P], identity[:]
                    )
                    nc.vector.tensor_copy(
                        vT[:, kcg // KC, kcg % KC, 0:64], tp[:, 0:64])
                    nc.vector.tensor_copy(
                        vT[:, kcg // KC, kcg % KC, 65:129], tp[:, 64:128])

            def emit_scores(b, qb, kcs, es_tiles):
                q0 = b * S + qb * QB
                k0 = b * S
                for kc in kcs:
                    sc = psum.tile([P, 2, QB], F32, name="sc", bufs=2)
                    for hi in range(2):
                        nc.tensor.matmul(
                            sc[:, hi, :],
                            qk[1][hi * D:(hi + 1) * D,
                                  k0 + kc * P:k0 + (kc + 1) * P],
                            qk[0][hi * D:(hi + 1) * D, q0:q0 + QB],
                            start=True, stop=True,
                            tile_position=(hi * D, 0),
                        )
                    if kc % EKC == 0:
                        es_tiles[kc // EKC] = es_pool.tile(
                            [P, EKC, 2, QB], BF16, name="es"
                        )
                    nc.scalar.activation(
                        es_tiles[kc // EKC][:, kc % EKC, :, :],
                        sc[:, :, :],
                        mybir.ActivationFunctionType.Exp,
                        scale=0.125,
                    )

            def emit_pv(b, qb, es_tiles, attn, vT):
                pv = [psum.tile([65, QB], F32, name=f"pv{hi}", bufs=1)
                      for hi in range(2)]
                for kc in range(KC):
                    for hi in range(2):
                        nc.tensor.matmul(
                            pv[hi][:],
                            vT[:, b, kc, hi * 65:(hi + 1) * 65],
                            es_tiles[kc // EKC][:, kc % EKC, hi, :],
                            start=(kc == 0),
                            stop=(kc == KC - 1),
                        )
                for hi in range(2):
                    inv = small.tile([1, QB], F32, name="inv")
                    nc.vector.reciprocal(inv[:], pv[hi][64:65, :])
                    inv_b = small.tile([D, QB], F32, name="invb")
                    nc.gpsimd.partition_broadcast(inv_b[:], inv[:], channels=D)
                    nc.vector.tensor_tensor(
                        out=attn[hi * D:(hi + 1) * D, :],
                        in0=pv[hi][0:64, :],
                        in1=inv_b[:],
                        op=mybir.AluOpType.mult,
                    )

            def emit_outproj(b, qb, attn):
                for tci in range(QB // P):
                    tc_g = (b * S + qb * QB + tci * P) // P
                    out_sb = outp.tile([P, E], F32, name="osb")
                    for eb in range(E // 512):
                        op = psum.tile([P, 512], F32, name="work", bufs=2)
                        nc.tensor.matmul(
                            op[:],
                            attn[:, tci * P:(tci + 1) * P],
                            w_outT_sb[:, eb * 512:(eb + 1) * 512],
                            start=True, stop=True,
                        )
                        nc.vector.tensor_copy(
                            out_sb[:, eb * 512:(eb + 1) * 512], op[:]
                        )
                    nc.sync.dma_start(out_d[tc_g], out_sb[:])

            def emit_qkv_head(vT):
                nc.vector.memset(vT[:, :, :, 64:65], 1.0)
                nc.vector.memset(vT[:, :, :, 129:130], 1.0)
                emit_tb(0, vT)
                emit_tb(1, vT)

            def emit_qkv_tail(vT, es2):
                emit_scores(0, 0, range(0, 8), es2)
                for tbi in range(2, NTB):
                    emit_tb(tbi, vT)

            # ---- prologue: repeat 0's QKV + first phase's scores ----
            phases = [(b, qb) for b in range(B) for qb in range(NQB)]
            vT_cur = vt_p.tile([P, B, KC, 130], BF16, name="vT")
            emit_qkv_head(vT_cur)
            es_next = {}
            emit_qkv_tail(vT_cur, es_next)
            emit_scores(0, 0, range(8, 16), es_next)

            for r in range(repeat):
                last = r == repeat - 1
                vT_nxt = None
                es_next2 = None
                for i, (b, qb) in enumerate(phases):
                    es_cur = es_next
                    es_next = {}
                    attn = attn_p.tile([P, QB], BF16, name="attn")
                    if i < 7:
                        nb, nqb = phases[i + 1]
                        emit_scores(nb, nqb, range(KC), es_next)
                        emit_pv(b, qb, es_cur, attn, vT_cur)
                        if i == 6 and not last:
                            vT_nxt = vt_p.tile([P, B, KC, 130], BF16, name="vT")
                            emit_qkv_head(vT_nxt)
                            emit_qkv_tail(vT_nxt, es_next2 := {})
                    else:
                        emit_pv(b, qb, es_cur, attn, vT_cur)
                        if not last:
                            emit_scores(0, 0, range(8, 16), es_next2)
                            es_next = es_next2
                            vT_cur = vT_nxt
                    emit_outproj(b, qb, attn)

    nc.compile()
    return nc


def _prep_inputs(x, w_in, b_in, w_out):
    x = np.ascontiguousarray(np.asarray(x, dtype=np.float32))
    w_in = np.asarray(w_in, dtype=np.float32)
    b_in = np.asarray(b_in, dtype=np.float32)
    w_out = np.asarray(w_out, dtype=np.float32)

    xT = np.ascontiguousarray(
        x.reshape(T, E).T.reshape(EC, P, T).transpose(1, 0, 2)
    ).astype(ml_dtypes.bfloat16)                # [128, EC, T]

    in_maps = []
    for c in range(NCORES):
        h0 = 2 * c
        rows = []
        brows = []
        for j in range(3):                      # q, k, v
            for hi in range(2):
                r0 = j * E + (h0 + hi) * D
                rows.append(w_in[r0:r0 + D])
                brows.append(b_in[r0:r0 + D])
        rows = np.concatenate(rows)             # [384, 1024]
        w_inT_c = np.ascontiguousarray(
            rows.T.reshape(EC, P, NCH * P).transpose(1, 0, 2)
        ).astype(ml_dtypes.bfloat16)            # [128, EC, 384]
        b_c = np.ascontiguousarray(
            np.concatenate(brows).reshape(NCH, P).T
        )                                       # [128, 3]
        # w_outT [128 = hi*64+d, E]
        w_outT_c = np.empty((P, E), dtype=np.float32)
        for hi in range(2):
            h = h0 + hi
            w_outT_c[hi * D:(hi + 1) * D] = w_out[:, h * D:(h + 1) * D].T
        w_outT_c = w_outT_c.astype(ml_dtypes.bfloat16)
        in_maps.append({
            "xT": xT,
            "w_inT": w_inT_c,
            "b_in": b_c,
            "w_outT": w_outT_c,
        })
    return in_maps


def kernel(x, w_in, b_in, w_out, b_out, _trace=False):
    global _COMPILED
    if _COMPILED is None:
        _COMPILED = build()
    nc = _COMPILED

    in_maps = _prep_inputs(x, w_in, b_in, w_out)
    res = run_bass_kernel_spmd(
        nc, in_maps, core_ids=list(range(NCORES)), trace=_trace
    )
    partial = np.zeros((T // P, P, E), dtype=np.float32)
    for c in range(NCORES):
        partial += res.results[c]["out"]
    out = partial.reshape(T, E) + np.asarray(b_out, dtype=np.float32)
    out = out.reshape(B, S, E)
    if _trace:
        return out, res
    return out
